# revision 19
# baseline (speedup 1.0000x reference)
"""Trainium2 Bass kernel for the 10-class supervised-contrastive loss.

Problem shapes (hardcoded): preds [10, 2048, 128] f32, target [2048] int64,
log_vars [10] f32 -> scalar f32.

The only O(B^2) quantity is Z[c, r] = sum_j exp(cos(r, j) / T); everything
else (P/R cosine sums via class feature sums, counts, log-prob assembly) is
O(B*D) / O(B*C) and computed on the host in f32.

Sharding (8 cores, SPMD, identical program per core; B=2048 -> 16 strips of
128 rows):
  - slot 0: core c owns class c's full upper trapezoid: strip a computes
    tiles (a, b) for b >= a (136 tiles).  exp symmetry: column sums of a
    computed tile (a,b) are the row sums of the skipped mirror tile (b,a);
    they accumulate per target strip in one PSUM bank (csum).
  - slot 1: classes 8 (cores 0-3) and 9 (cores 4-7) are split 4 ways, 34
    tiles each, with an IDENTICAL static program: fed-coordinate tiles
      row 0: cols 0..9 | row 1: cols 1..9 | row 8: cols 8..15 | row 9: 9..15
    Core j feeds the class's features rotated by 2j strips (np.roll by
    256*j rows).  The 4 rotated images of this 34-tile set partition the
    class's 136 unordered strip pairs exactly (positions {0,1,8,9} tile
    Z16 under shifts {0,2,4,6}; {0,1} tiles Z8 for the distance-8 band).
    Each core outputs raw partials (direct row sums + csum mirror sums);
    the host sums partials across the 4 cores.

Device pipeline per unit (row strip): bf16 matmuls C = G_a^T G (f32 PSUM,
512-col chunks into a [128,1536] 3-bank cp tile), DVE zeroes the diagonal
window, one wide ACT Exp(C/T) -> bf16 sc with fused accum_out row sum, PE
1-col matmuls (sc tile stationary) accumulate mirror column sums.

Host epilogue: Z = direct + mirror - 1 (diag contributed exp(0)=1), masked
mean log-prob from host P/R + analytic counts, uncertainty-weighted sum.
"""

import ml_dtypes
import numpy as np

import concourse.bacc as bacc
import concourse.bass as bass
import concourse.mybir as mybir
import concourse.tile as tile
from concourse.bass_utils import run_bass_kernel_spmd

NUM_CLASSES = 10
B = 2048
D = 128
T = 0.07
BASE_T = 0.07
N_CORES = 8

f32 = mybir.dt.float32
bf16 = mybir.dt.bfloat16
np_bf16 = ml_dtypes.bfloat16

# Slot-1 static units: (fed_row, col_start, col_end).
S1 = [(0, 0, 1280), (1, 128, 1280), (8, 1024, 2048), (9, 1152, 2048)]

# Unit order (mirror sums accumulate in SBUF, so any order works): start
# with the narrow high strips (they only need the tail G chunks, which are
# DMA'd first, so ACT ramps ~3us earlier), wide strips mid-stream, slot-1
# units (encoded (1, k)) once their G has landed, and end on the narrowest
# strip for a short tail.
ORDER = [
    (0, 14), (0, 13), (0, 12), (0, 11), (0, 10), (0, 9), (0, 8),
    (0, 0), (1, 0), (0, 1), (1, 1), (0, 2), (1, 2), (0, 3), (1, 3),
    (0, 4), (0, 5), (0, 6), (0, 7), (0, 15),
]

# Output columns (f32): 0..15 slot-0 direct row sums; 16..19 slot-1 direct
# row sums (fed rows 0,1,8,9); 20..34 slot-1 csum for fed strips 1..15;
# 35..49 slot-0 csum for strips 1..15; 50..51 scratch accumulators.
OUT_W = 52

TRACE = False
LAST_RESULT = None


def _chunks(c0, c1):
    """Split [c0, c1) at 512-aligned boundaries (PSUM bank limit)."""
    out = []
    c = c0
    while c < c1:
        nxt = min(c1, (c // 512 + 1) * 512)
        out.append((c, nxt))
        c = nxt
    return out


def _build_nc():
    nc = bacc.Bacc(None, target_bir_lowering=False)

    masknd_dram = nc.dram_tensor("masknd", [128, 128], f32, kind="ExternalInput")
    onesf_dram = nc.dram_tensor("onesf", [128, 1], bf16, kind="ExternalInput")
    g_dram = [
        [
            nc.dram_tensor(f"g{s}c{k}", [128, 512], bf16, kind="ExternalInput")
            for k in range(4)
        ]
        for s in range(2)
    ]
    out_dram = nc.dram_tensor("out", [128, OUT_W], f32, kind="ExternalOutput")

    add = mybir.AluOpType.add
    EXP = mybir.ActivationFunctionType.Exp

    with tile.TileContext(nc) as tc:
        with (
            tc.tile_pool(name="const", bufs=1) as constp,
            tc.tile_pool(name="gmat", bufs=1) as gmatp,
            tc.tile_pool(name="scp", bufs=4) as scp,
        ):
            # Exp-table preload: a dummy ACTIVATE on an uninitialized scratch
            # tile (no producer, so no wait) pulls the ~1.3us ACT table load
            # into the DMA window instead of the first real call.
            warm = constp.tile([128, 2], bf16, tag="warm")
            nc.scalar.activation(
                warm[:, 1:2], warm[:, 0:1], mybir.ActivationFunctionType.Exp
            )
            # g0 chunks tail-first on the sync queue (the first units only
            # need the high chunks), g1 on the scalar queue (parallel
            # descriptor issue), tiny constants on the idle gpsimd queue.
            G = [[None] * 4 for _ in range(2)]
            for k in (3, 2, 1, 0):
                g = gmatp.tile([128, 512], bf16, tag=f"G0c{k}", name=f"G0c{k}")
                nc.sync.dma_start(g[:], g_dram[0][k][:])
                G[0][k] = g
            masknd_sb = constp.tile([128, 128], f32, tag="masknd")
            nc.gpsimd.dma_start(masknd_sb[:], masknd_dram[:])
            onesf_sb = constp.tile([128, 1], bf16, tag="onesf")
            nc.gpsimd.dma_start(onesf_sb[:], onesf_dram[:])
            for k in range(4):
                g = gmatp.tile([128, 512], bf16, tag=f"G1c{k}", name=f"G1c{k}")
                nc.scalar.dma_start(g[:], g_dram[1][k][:])
                G[1][k] = g
            out_sb = constp.tile([128, OUT_W], f32, tag="out")

            def strip(s, rb):
                k = rb // 4
                o = (rb % 4) * 128
                return G[s][k][:, o : o + 128]

            def gcols(s, a0, a1):
                k = a0 // 512
                return G[s][k][:, a0 - 512 * k : a1 - 512 * k]

            # Mirror row-sum accumulator in SBUF: col cb (slot 0) / 16+cb
            # (slot 1).  PSUM accumulation groups held open across other
            # matmuls in the same bank are silently invalidated by later
            # start=True writes, so every csum matmul below is a CLOSED
            # (start+stop) write into a per-unit PSUM tile and the running
            # sums live here, maintained by one DVE add per unit.
            mir_sb = constp.tile([128, 32], f32, tag="mir")
            nc.vector.memset(mir_sb[:], 0.0)

            with tc.tile_pool(name="mainpsum", bufs=2, space="PSUM") as cpp:
                ones_col = onesf_sb[:]  # [128,1] bf16 ones

                for s, idx in ORDER:
                    if s == 0:
                        rb = idx
                        c0, c1 = rb * 128, 2048
                        acol = out_sb[:, rb : rb + 1]
                        regions = (
                            [(0, c0, 1024), (1024, 1024, 2048)]
                            if rb < 4
                            else [(512 * (c0 // 512), c0, 2048)]
                        )
                        ccol = 0  # csum column base for slot 0
                    else:
                        rb, c0, c1 = S1[idx]
                        acol = out_sb[:, 16 + idx : 17 + idx]
                        regions = [(512 * (c0 // 512), c0, c1)]
                        ccol = 16
                    lhsT = strip(s, rb)
                    acc2 = None
                    cs = cpp.tile([128, 16], f32, tag="cs", name=f"cs{s}_{idx}")
                    cb_lo, cb_hi = 16, 0
                    for base, r0, r1 in regions:
                        cp = cpp.tile(
                            [128, 1536], f32, tag="cp", name=f"cp{s}_{idx}_{base}"
                        )
                        for a0, a1 in _chunks(r0, r1):
                            nc.tensor.matmul(
                                cp[:, a0 - base : a1 - base],
                                lhsT,
                                gcols(s, a0, a1),
                                start=True,
                                stop=True,
                            )
                        if r0 <= rb * 128 < r1:
                            # Zero the diagonal window.
                            w0 = rb * 128 - base
                            nc.vector.tensor_mul(
                                cp[:, w0 : w0 + 128],
                                cp[:, w0 : w0 + 128],
                                masknd_sb[:],
                            )
                        sc = scp.tile(
                            [128, 1536], bf16, tag="sc", name=f"sc{s}_{idx}_{base}"
                        )
                        tgt = acol
                        if base == 1024 and s == 0 and rb < 4:
                            # Second region accumulates into a scratch col.
                            tgt = out_sb[:, 50 + (rb % 2) : 51 + (rb % 2)]
                            acc2 = tgt
                        nc.scalar.activation(
                            sc[:, r0 - base : r1 - base],
                            cp[:, r0 - base : r1 - base],
                            EXP,
                            scale=1.0 / T,
                            accum_out=tgt,
                        )
                        # Column sums of computed tiles -> mirror row sums.
                        for cb in range(max(rb + 1, (r0 + 127) // 128), r1 // 128):
                            nc.tensor.matmul(
                                cs[:, cb : cb + 1],
                                sc[:, cb * 128 - base : cb * 128 - base + 128],
                                ones_col,
                                start=True,
                                stop=True,
                            )
                            cb_lo = min(cb_lo, cb)
                            cb_hi = max(cb_hi, cb + 1)
                    if acc2 is not None:
                        nc.vector.tensor_tensor(
                            out=acol, in0=acol, in1=acc2, op=add
                        )
                    # Fold this unit's closed column-sum partials into the
                    # running SBUF mirror accumulator.
                    if cb_hi > cb_lo:
                        dst = mir_sb[:, ccol + cb_lo : ccol + cb_hi]
                        nc.vector.tensor_tensor(
                            out=dst, in0=dst, in1=cs[:, cb_lo:cb_hi], op=add
                        )

                # Ship raw mirror partials; host adds them to direct sums.
                nc.vector.tensor_copy(out_sb[:, 20:35], mir_sb[:, 17:32])
                nc.vector.tensor_copy(out_sb[:, 35:50], mir_sb[:, 1:16])

            nc.sync.dma_start(out_dram[:, 0:50], out_sb[:, 0:50])
    nc.finalize()
    return nc


_NC_CACHE = None


def _get_nc():
    global _NC_CACHE
    if _NC_CACHE is None:
        _NC_CACHE = _build_nc()
    return _NC_CACHE


def kernel(preds, target, log_vars):
    global LAST_RESULT
    preds = np.asarray(preds, dtype=np.float32)
    target = np.asarray(target)
    log_vars = np.asarray(log_vars, dtype=np.float32)

    onehot = (target[None, :] == np.arange(NUM_CLASSES, dtype=target.dtype)[:, None])
    onehot = onehot.astype(np.float32)  # [10, B]
    npos = onehot.sum(axis=1).astype(np.float64)  # [10]

    # Host prep: row-normalize (f32 stats), cast bf16, d-major layout.
    norms = np.sqrt((preds.astype(np.float32) ** 2).sum(axis=2, dtype=np.float32))
    ghat32 = preds / norms[:, :, None]  # [10, B, D] f32
    ghat = ghat32.astype(np_bf16)

    # Host P/R: per-row cosine sums against positives / all rows (f32).
    u_all = ghat32.sum(axis=1)  # [10, D]
    u_pos = np.einsum("cbd,cb->cd", ghat32, onehot)  # [10, D]
    P = np.einsum("cbd,cd->cb", ghat32, u_pos)  # [10, B]
    R = np.einsum("cbd,cd->cb", ghat32, u_all)  # [10, B]

    masknd = np.ascontiguousarray(1.0 - np.eye(128, dtype=np.float32))

    in_maps = []
    for c in range(N_CORES):
        cls1 = 8 + c // 4
        off = 256 * (c % 4)  # rotation: fed strip f = actual strip f + 2j
        im = {"masknd": masknd, "onesf": np.ones((128, 1), np_bf16)}
        for s, (cls, o) in enumerate([(c, 0), (cls1, off)]):
            gh = np.roll(ghat[cls], -o, axis=0) if o else ghat[cls]
            gt = np.ascontiguousarray(gh.T)  # [128, 2048] [d, b]
            for k in range(4):
                im[f"g{s}c{k}"] = np.ascontiguousarray(gt[:, 512 * k : 512 * (k + 1)])
        in_maps.append(im)

    nc = _get_nc()
    res = run_bass_kernel_spmd(nc, in_maps, list(range(N_CORES)), trace=TRACE)
    LAST_RESULT = res

    # Assemble Z (sum over j != i of exp(cos_ij / T)) from partials.
    outs = [np.asarray(res.results[c]["out"], dtype=np.float64) for c in range(N_CORES)]
    Z = np.zeros((NUM_CLASSES, B), dtype=np.float64)
    for c in range(N_CORES):
        o = outs[c]
        # Slot 0: own class, strips 0..15: direct + mirror csum - diag.
        for rb in range(16):
            z = o[:, rb].copy()
            if rb > 0:
                z += o[:, 35 + rb - 1]
            Z[c, rb * 128 : rb * 128 + 128] = z - 1.0
    fed_direct = {0: 16, 1: 17, 8: 18, 9: 19}
    for cls in (8, 9):
        cores = range(0, 4) if cls == 8 else range(4, 8)
        for t in range(16):
            acc = np.zeros(128, dtype=np.float64)
            for c in cores:
                j = c % 4
                f = (t - 2 * j) % 16
                if f in fed_direct:
                    acc += outs[c][:, fed_direct[f]]
                if f >= 1:
                    acc += outs[c][:, 20 + f - 1]
            Z[cls, t * 128 : t * 128 + 128] = acc - 1.0

    lab = onehot.astype(np.float64)
    masked_cos = lab * P.astype(np.float64) + (1.0 - lab) * (R - P).astype(np.float64)
    masked_logits_sum = (masked_cos - 1.0) / T
    cnt = lab * npos[:, None] + (1.0 - lab) * (B - npos[:, None]) - 1.0
    mlpp = masked_logits_sum / cnt - np.log(Z)
    losses = -(T / BASE_T) * mlpp.mean(axis=1)  # [10]
    lv = log_vars.astype(np.float64)
    final = np.sum(np.exp(-lv) * losses + lv)
    return np.float32(final)


# revision 21
# speedup vs baseline: 1.1419x; 1.1419x over previous
"""Trainium2 Bass kernel for the 10-class supervised-contrastive loss.

Problem shapes (hardcoded): preds [10, 2048, 128] f32, target [2048] int64,
log_vars [10] f32 -> scalar f32.

The only O(B^2) quantity is Z[c, r] = sum_j exp(cos(r, j) / T); everything
else (P/R cosine sums via class feature sums, counts, log-prob assembly) is
O(B*D) / O(B*C) and computed on the host in f32.

Sharding (8 cores, SPMD, identical program per core; B=2048 -> 16 strips of
128 rows):
  - slot 0: core c owns class c's full upper trapezoid: strip a computes
    tiles (a, b) for b >= a (136 tiles).  exp symmetry: column sums of a
    computed tile (a,b) are the row sums of the skipped mirror tile (b,a);
    they accumulate per target strip in one PSUM bank (csum).
  - slot 1: classes 8 (cores 0-3) and 9 (cores 4-7) are split 4 ways, 34
    tiles each, with an IDENTICAL static program: fed-coordinate tiles
      row 0: cols 0..9 | row 1: cols 1..9 | row 8: cols 8..15 | row 9: 9..15
    Core j feeds the class's features rotated by 2j strips (np.roll by
    256*j rows).  The 4 rotated images of this 34-tile set partition the
    class's 136 unordered strip pairs exactly (positions {0,1,8,9} tile
    Z16 under shifts {0,2,4,6}; {0,1} tiles Z8 for the distance-8 band).
    Each core outputs raw partials (direct row sums + csum mirror sums);
    the host sums partials across the 4 cores.

Device pipeline per unit (row strip): bf16 matmuls C = G_a^T G (f32 PSUM,
512-col chunks into a [128,1536] 3-bank cp tile), DVE zeroes the diagonal
window, one wide ACT Exp(C/T) -> bf16 sc with fused accum_out row sum, PE
1-col matmuls (sc tile stationary) accumulate mirror column sums.

Host epilogue: Z = direct + mirror - 1 (diag contributed exp(0)=1), masked
mean log-prob from host P/R + analytic counts, uncertainty-weighted sum.
"""

import ml_dtypes
import numpy as np

import concourse.bacc as bacc
import concourse.bass as bass
import concourse.mybir as mybir
import concourse.tile as tile
from concourse.bass_utils import run_bass_kernel_spmd

NUM_CLASSES = 10
B = 2048
D = 128
T = 0.07
BASE_T = 0.07
N_CORES = 8

f32 = mybir.dt.float32
bf16 = mybir.dt.bfloat16
np_bf16 = ml_dtypes.bfloat16

# Slot-1 static units: (fed_row, col_start, col_end).
S1 = [(0, 0, 1280), (1, 128, 1280), (8, 1024, 2048), (9, 1152, 2048)]

# Unit order (mirror sums accumulate in SBUF, so any order works): wide
# strips first keep the ACT pipeline fed during the ramp, slot-1 units
# (encoded (1, k)) interleave once their G has landed, narrow strips last
# for a short tail.
ORDER = [
    (0, 0), (0, 1), (0, 2), (0, 3),
    (1, 0), (0, 4), (1, 1), (0, 5), (0, 6), (1, 2), (0, 7), (0, 8),
    (1, 3), (0, 9), (0, 10), (0, 11), (0, 12), (0, 13), (0, 14), (0, 15),
]

# Output columns (f32): 0..15 slot-0 direct row sums; 16..19 slot-1 direct
# row sums (fed rows 0,1,8,9); 20..34 slot-1 csum for fed strips 1..15;
# 35..49 slot-0 csum for strips 1..15; 50..51 scratch accumulators.
OUT_W = 52

TRACE = False
LAST_RESULT = None


def _chunks(c0, c1):
    """Split [c0, c1) at 512-aligned boundaries (PSUM bank limit)."""
    out = []
    c = c0
    while c < c1:
        nxt = min(c1, (c // 512 + 1) * 512)
        out.append((c, nxt))
        c = nxt
    return out


def _build_nc():
    nc = bacc.Bacc(None, target_bir_lowering=False)

    masknd_dram = nc.dram_tensor("masknd", [128, 128], f32, kind="ExternalInput")
    onesf_dram = nc.dram_tensor("onesf", [128, 1], bf16, kind="ExternalInput")
    g_dram = [
        [
            nc.dram_tensor(f"g{s}c{k}", [128, 512], bf16, kind="ExternalInput")
            for k in range(4)
        ]
        for s in range(2)
    ]
    out_dram = nc.dram_tensor("out", [128, OUT_W], f32, kind="ExternalOutput")

    add = mybir.AluOpType.add
    EXP = mybir.ActivationFunctionType.Exp

    with tile.TileContext(nc) as tc:
        with (
            tc.tile_pool(name="const", bufs=1) as constp,
            tc.tile_pool(name="gmat", bufs=1) as gmatp,
            tc.tile_pool(name="scp", bufs=4) as scp,
        ):
            # Exp-table preload: a dummy ACTIVATE on an uninitialized scratch
            # tile (no producer, so no wait) pulls the ~1.3us ACT table load
            # into the DMA window instead of the first real call.
            warm = constp.tile([128, 2], bf16, tag="warm")
            nc.scalar.activation(
                warm[:, 1:2], warm[:, 0:1], mybir.ActivationFunctionType.Exp
            )
            # g0 chunks tail-first on the sync queue (the first units only
            # need the high chunks), g1 on the scalar queue (parallel
            # descriptor issue), tiny constants on the idle gpsimd queue.
            G = [[None] * 4 for _ in range(2)]
            for k in range(4):
                g = gmatp.tile([128, 512], bf16, tag=f"G0c{k}", name=f"G0c{k}")
                nc.sync.dma_start(g[:], g_dram[0][k][:])
                G[0][k] = g
            masknd_sb = constp.tile([128, 128], f32, tag="masknd")
            nc.gpsimd.dma_start(masknd_sb[:], masknd_dram[:])
            onesf_sb = constp.tile([128, 1], bf16, tag="onesf")
            nc.gpsimd.dma_start(onesf_sb[:], onesf_dram[:])
            for k in range(4):
                g = gmatp.tile([128, 512], bf16, tag=f"G1c{k}", name=f"G1c{k}")
                nc.scalar.dma_start(g[:], g_dram[1][k][:])
                G[1][k] = g
            out_sb = constp.tile([128, OUT_W], f32, tag="out")

            def strip(s, rb):
                k = rb // 4
                o = (rb % 4) * 128
                return G[s][k][:, o : o + 128]

            def gcols(s, a0, a1):
                k = a0 // 512
                return G[s][k][:, a0 - 512 * k : a1 - 512 * k]

            # Mirror row-sum accumulator in SBUF: col cb (slot 0) / 16+cb
            # (slot 1).  PSUM accumulation groups held open across other
            # matmuls in the same bank are silently invalidated by later
            # start=True writes, so every csum matmul below is a CLOSED
            # (start+stop) write into a per-unit PSUM tile and the running
            # sums live here, maintained by one DVE add per unit.
            mir_sb = constp.tile([128, 32], f32, tag="mir")
            nc.vector.memset(mir_sb[:], 0.0)

            with tc.tile_pool(name="mainpsum", bufs=2, space="PSUM") as cpp:
                ones_col = onesf_sb[:]  # [128,1] bf16 ones

                for s, idx in ORDER:
                    if s == 0:
                        rb = idx
                        c0, c1 = rb * 128, 2048
                        acol = out_sb[:, rb : rb + 1]
                        regions = (
                            [(0, c0, 1024), (1024, 1024, 2048)]
                            if rb < 4
                            else [(512 * (c0 // 512), c0, 2048)]
                        )
                        ccol = 0  # csum column base for slot 0
                    else:
                        rb, c0, c1 = S1[idx]
                        acol = out_sb[:, 16 + idx : 17 + idx]
                        regions = [(512 * (c0 // 512), c0, c1)]
                        ccol = 16
                    lhsT = strip(s, rb)
                    acc2 = None
                    cs = cpp.tile([128, 16], f32, tag="cs", name=f"cs{s}_{idx}")
                    cb_lo, cb_hi = 16, 0
                    for base, r0, r1 in regions:
                        cp = cpp.tile(
                            [128, 1536], f32, tag="cp", name=f"cp{s}_{idx}_{base}"
                        )
                        for a0, a1 in _chunks(r0, r1):
                            nc.tensor.matmul(
                                cp[:, a0 - base : a1 - base],
                                lhsT,
                                gcols(s, a0, a1),
                                start=True,
                                stop=True,
                            )
                        if r0 <= rb * 128 < r1:
                            # Zero the diagonal window.
                            w0 = rb * 128 - base
                            nc.vector.tensor_mul(
                                cp[:, w0 : w0 + 128],
                                cp[:, w0 : w0 + 128],
                                masknd_sb[:],
                            )
                        sc = scp.tile(
                            [128, 1536], bf16, tag="sc", name=f"sc{s}_{idx}_{base}"
                        )
                        tgt = acol
                        if base == 1024 and s == 0 and rb < 4:
                            # Second region accumulates into a scratch col.
                            tgt = out_sb[:, 50 + (rb % 2) : 51 + (rb % 2)]
                            acc2 = tgt
                        nc.scalar.activation(
                            sc[:, r0 - base : r1 - base],
                            cp[:, r0 - base : r1 - base],
                            EXP,
                            scale=1.0 / T,
                            accum_out=tgt,
                        )
                        # Column sums of computed tiles -> mirror row sums.
                        for cb in range(max(rb + 1, (r0 + 127) // 128), r1 // 128):
                            nc.tensor.matmul(
                                cs[:, cb : cb + 1],
                                sc[:, cb * 128 - base : cb * 128 - base + 128],
                                ones_col,
                                start=True,
                                stop=True,
                            )
                            cb_lo = min(cb_lo, cb)
                            cb_hi = max(cb_hi, cb + 1)
                    if acc2 is not None:
                        nc.vector.tensor_tensor(
                            out=acol, in0=acol, in1=acc2, op=add
                        )
                    # Fold this unit's closed column-sum partials into the
                    # running SBUF mirror accumulator.
                    if cb_hi > cb_lo:
                        dst = mir_sb[:, ccol + cb_lo : ccol + cb_hi]
                        nc.vector.tensor_tensor(
                            out=dst, in0=dst, in1=cs[:, cb_lo:cb_hi], op=add
                        )

                # Ship raw mirror partials; host adds them to direct sums.
                nc.vector.tensor_copy(out_sb[:, 20:35], mir_sb[:, 17:32])
                nc.vector.tensor_copy(out_sb[:, 35:50], mir_sb[:, 1:16])

            nc.sync.dma_start(out_dram[:, 0:50], out_sb[:, 0:50])
    nc.finalize()
    return nc


_NC_CACHE = None


def _get_nc():
    global _NC_CACHE
    if _NC_CACHE is None:
        _NC_CACHE = _build_nc()
    return _NC_CACHE


def kernel(preds, target, log_vars):
    global LAST_RESULT
    preds = np.asarray(preds, dtype=np.float32)
    target = np.asarray(target)
    log_vars = np.asarray(log_vars, dtype=np.float32)

    onehot = (target[None, :] == np.arange(NUM_CLASSES, dtype=target.dtype)[:, None])
    onehot = onehot.astype(np.float32)  # [10, B]
    npos = onehot.sum(axis=1).astype(np.float64)  # [10]

    # Host prep: row-normalize (f32 stats), cast bf16, d-major layout.
    norms = np.sqrt((preds.astype(np.float32) ** 2).sum(axis=2, dtype=np.float32))
    ghat32 = preds / norms[:, :, None]  # [10, B, D] f32
    ghat = ghat32.astype(np_bf16)

    # Host P/R: per-row cosine sums against positives / all rows (f32).
    u_all = ghat32.sum(axis=1)  # [10, D]
    u_pos = np.einsum("cbd,cb->cd", ghat32, onehot)  # [10, D]
    P = np.einsum("cbd,cd->cb", ghat32, u_pos)  # [10, B]
    R = np.einsum("cbd,cd->cb", ghat32, u_all)  # [10, B]

    masknd = np.ascontiguousarray(1.0 - np.eye(128, dtype=np.float32))

    in_maps = []
    for c in range(N_CORES):
        cls1 = 8 + c // 4
        off = 256 * (c % 4)  # rotation: fed strip f = actual strip f + 2j
        im = {"masknd": masknd, "onesf": np.ones((128, 1), np_bf16)}
        for s, (cls, o) in enumerate([(c, 0), (cls1, off)]):
            gh = np.roll(ghat[cls], -o, axis=0) if o else ghat[cls]
            gt = np.ascontiguousarray(gh.T)  # [128, 2048] [d, b]
            for k in range(4):
                im[f"g{s}c{k}"] = np.ascontiguousarray(gt[:, 512 * k : 512 * (k + 1)])
        in_maps.append(im)

    nc = _get_nc()
    res = run_bass_kernel_spmd(nc, in_maps, list(range(N_CORES)), trace=TRACE)
    LAST_RESULT = res

    # Assemble Z (sum over j != i of exp(cos_ij / T)) from partials.
    outs = [np.asarray(res.results[c]["out"], dtype=np.float64) for c in range(N_CORES)]
    Z = np.zeros((NUM_CLASSES, B), dtype=np.float64)
    for c in range(N_CORES):
        o = outs[c]
        # Slot 0: own class, strips 0..15: direct + mirror csum - diag.
        for rb in range(16):
            z = o[:, rb].copy()
            if rb > 0:
                z += o[:, 35 + rb - 1]
            Z[c, rb * 128 : rb * 128 + 128] = z - 1.0
    fed_direct = {0: 16, 1: 17, 8: 18, 9: 19}
    for cls in (8, 9):
        cores = range(0, 4) if cls == 8 else range(4, 8)
        for t in range(16):
            acc = np.zeros(128, dtype=np.float64)
            for c in cores:
                j = c % 4
                f = (t - 2 * j) % 16
                if f in fed_direct:
                    acc += outs[c][:, fed_direct[f]]
                if f >= 1:
                    acc += outs[c][:, 20 + f - 1]
            Z[cls, t * 128 : t * 128 + 128] = acc - 1.0

    lab = onehot.astype(np.float64)
    masked_cos = lab * P.astype(np.float64) + (1.0 - lab) * (R - P).astype(np.float64)
    masked_logits_sum = (masked_cos - 1.0) / T
    cnt = lab * npos[:, None] + (1.0 - lab) * (B - npos[:, None]) - 1.0
    mlpp = masked_logits_sum / cnt - np.log(Z)
    losses = -(T / BASE_T) * mlpp.mean(axis=1)  # [10]
    lv = log_vars.astype(np.float64)
    final = np.sum(np.exp(-lv) * losses + lv)
    return np.float32(final)
